# revision 37
# baseline (speedup 1.0000x reference)
"""Trainium2 Bass kernel for nn_MultiScaleGeometricAttention.

Reference semantics (ages=0 => attention_scale = 0.05):
    eff_t[n] = (|temperature[n]| + 0.1) * 0.05
    q[r, n]  = ||x_r||^2 + ||p_n||^2 - 2 * (x_r . p_n)
    d = sqrt(q);   w = exp(-d / eff_t)
    out = (w @ values) / (w @ 1 + 1e-8)
(the per-row normalization commutes with the value GEMM, so it is applied
after both GEMMs)

Sharding: data-parallel over flattened B*T rows; 2048 rows per core on 8
cores; positions/values/temperature replicated.

Per-core device pipeline (layout S^T: n on partitions, rows on free axis):
    GEMM1 (PE, fp8 DoubleRow):  psq[n, r] = (-2 p)^T @ x   (f32 PSUM, K=512)
    DVE:   q = (psq + p2[n]) + x2[r]     (one scalar_tensor_tensor, bf16 out)
    ACT:   d = sqrt(q)                   (one batched activation per group)
    ACT:   w = exp(-d/t[n]) -> fp8       (per-j, per-partition scale)
    GEMM2 (PE, fp8 DoubleRow, paced per w-pair so the PE never sees a
           full HAM idle window):  o[r,:] += w_pair^T @ (64 v);
                                   s[r]  += ones^T @ w_pair
    PE:    transpose (s + eps) to partitions via K=1 matmuls (x64)
    DVE:   out = o * (1 / (64 (s + 1e-8)))
"""

import sys

if "/opt/trn_rl_repo" not in sys.path:
    sys.path.insert(0, "/opt/trn_rl_repo")

import numpy as np
import ml_dtypes

P = 128
CHUNK = 512  # row-columns of S^T processed per chunk (PSUM free dim)
GROUP = 32   # n-tiles per sqrt batch group

N_CORES = 8


def build_program(R=2048, N=4096, D=512, uniform_nit=None):
    import concourse.mybir as mybir
    import concourse.tile as tile
    from concourse import bacc

    f32 = mybir.dt.float32
    bf16 = mybir.dt.bfloat16
    f8 = mybir.dt.float8e4
    DR = mybir.MatmulPerfMode.DoubleRow
    Alu = mybir.AluOpType
    Act = mybir.ActivationFunctionType

    KT = D // P      # contraction tiles for GEMM1
    NT = N // P      # n tiles
    CH = R // CHUNK  # chunks
    BTT = CHUNK // P # row tiles per chunk
    GRP = min(GROUP, NT)
    NG = NT // GRP   # groups per chunk

    nc = bacc.Bacc()
    xT = nc.declare_dram_parameter("xT", [D, R], f8, isOutput=False)
    posTm2 = nc.declare_dram_parameter("posTm2", [D, N], f8, isOutput=False)
    v = nc.declare_dram_parameter("v", [N, D], f8, isOutput=False)
    # aux: [:, :R] = x2 broadcast, [:, R:R+NT] = p2, [:, R+NT:] = -1/eff_t
    aux = nc.declare_dram_parameter("aux", [P, R + 2 * NT], f32, isOutput=False)
    out = nc.declare_dram_parameter("out", [R, D], f32, isOutput=True)

    with tile.TileContext(nc) as tc:
        with (
            tc.tile_pool(name="singles", bufs=1) as singles,
            tc.tile_pool(name="xt", bufs=2) as xt_pool,
            tc.tile_pool(name="q", bufs=5) as q_pool,
            tc.tile_pool(name="w", bufs=3) as w_pool,
            tc.tile_pool(name="o", bufs=4) as o_pool,
            tc.tile_pool(name="sr", bufs=2) as sr_pool,
            tc.tile_pool(name="psq", bufs=2, space="PSUM") as psq_pool,
            tc.tile_pool(name="pso", bufs=1, space="PSUM") as pso_pool,
            tc.tile_pool(name="psr", bufs=1, space="PSUM") as psr_pool,
            tc.tile_pool(name="rpt", bufs=1, space="PSUM") as rpt_pool,
        ):
            xT_r = xT[:, :].rearrange("(kt p) r -> p kt r", p=P)
            xts = [None] * CH

            def load_xt(c):
                t = xt_pool.tile([P, KT, CHUNK], f8, tag="xt", name=f"xt{c}")
                nc.sync.dma_start(
                    out=t, in_=xT_r[:, :, c * CHUNK : (c + 1) * CHUNK]
                )
                xts[c] = t

            # first chunk of x lands first so GEMM1 can start ASAP
            load_xt(0)
            # the DVE/ACT chain needs p2/nit + chunk-0 x2 right away: put
            # those small slices first on the sync ring
            aux_sb = singles.tile([P, R + 2 * NT], f32)
            nc.sync.dma_start(
                out=aux_sb[:, R : R + 2 * NT], in_=aux[:, R : R + 2 * NT]
            )
            nc.sync.dma_start(out=aux_sb[:, :CHUNK], in_=aux[:, :CHUNK])
            # positions as k-PAIR tiles (DoubleRow lhsT layout [Ki, 2, M])
            posTm2_r = posTm2[:, :].rearrange("(kp s p) n -> kp p s n", p=P, s=2)
            posT_tiles = []
            dma_engines = [nc.sync, nc.gpsimd]
            nsplit = max(P, N // 4)
            for kp in range(KT // 2):
                pt = singles.tile([P, 2, N], f8, name=f"posT{kp}")
                eng = dma_engines[kp % len(dma_engines)]
                # low-j columns land first so GEMM1 can start sooner
                eng.dma_start(out=pt[:, :, :nsplit], in_=posTm2_r[kp][:, :, :nsplit])
                posT_tiles.append(pt)
            for kp in range(KT // 2):
                eng = dma_engines[kp % len(dma_engines)]
                eng.dma_start(
                    out=posT_tiles[kp][:, :, nsplit:],
                    in_=posTm2_r[kp][:, :, nsplit:],
                )
            if R > CHUNK:
                nc.gpsimd.dma_start(
                    out=aux_sb[:, CHUNK:R], in_=aux[:, CHUNK:R]
                )
            x2b_sb = aux_sb[:, :R]
            p2_sb = aux_sb[:, R : R + NT]
            nit_sb = aux_sb[:, R + NT : R + 2 * NT]
            v_sb = singles.tile([P, NT, D], f8)
            v_r = v[:, :].rearrange("(vh nt p) d -> vh p nt d", p=P, vh=4)
            v_engines = [nc.gpsimd, nc.gpsimd, nc.sync, nc.sync]
            for h in range(4):
                v_engines[h].dma_start(
                    out=v_sb[:, h * (NT // 4) : (h + 1) * (NT // 4), :], in_=v_r[h]
                )
            # padded so the DoubleRow K-pair stride is 16 (ISA constraint)
            ones2_sb_full = singles.tile([P, 2, 16], f8)
            nc.vector.memset(ones2_sb_full, 1.0)
            ones2_sb = ones2_sb_full[:, :, 0:1]
            # 64.0 compensates the host-side v*64 fp8 scaling:
            # rpt = 64*(s+eps); out = (64*w@v) / (64*(s+eps))
            one1_sb = singles.tile([1, 1], f32)
            nc.vector.memset(one1_sb, 64.0)
            eps_sb = singles.tile([1, CHUNK], f32)
            nc.vector.memset(eps_sb, 1e-8)

            pending_drain = [None]
            for c in range(CH):
                c0 = c * CHUNK
                if c + 1 < CH:
                    load_xt(c + 1)
                xt = xts[c]

                pso_tiles = [
                    pso_pool.tile([P, D], f32, tag=f"pso{i}", name=f"pso{i}")
                    for i in range(BTT)
                ]
                srow_ps = psr_pool.tile([1, CHUNK], f32, tag="psrow")
                q_tiles = [None] * NG

                QB = 8  # j's per sqrt/exp batch (block tiles)

                def emit_g1(g, xt=xt, c0=c0, q_tiles=q_tiles):
                    # GEMM1 (fp8 DR) + q assembly + quarter-batched sqrts
                    # (separate tiles -> fine-grained deps, the first exp
                    # can start after only QB columns of GEMM1)
                    quarters = []
                    for jj in range(GRP):
                        j = g * GRP + jj
                        if jj % QB == 0:
                            q_q = q_pool.tile(
                                [P, QB, CHUNK], bf16, tag="q", name="q_q"
                            )
                            quarters.append(q_q)
                        psq = psq_pool.tile([P, CHUNK], f32, tag="psq", name="psq")
                        for kp in range(KT // 2):
                            nc.tensor.matmul(
                                psq,
                                posT_tiles[kp][:, :, j * P : (j + 1) * P],
                                xt[:, 2 * kp : 2 * kp + 2, :],
                                perf_mode=DR,
                                start=(kp == 0),
                                stop=(kp == KT // 2 - 1),
                            )
                        # q = (psq + p2[n]) + x2[r]
                        nc.vector.scalar_tensor_tensor(
                            out=q_q[:, jj % QB, :],
                            in0=psq,
                            scalar=p2_sb[:, j : j + 1],
                            in1=x2b_sb[:, c0 : c0 + CHUNK],
                            op0=Alu.add,
                            op1=Alu.add,
                        )
                        if jj % QB == QB - 1:
                            # d = sqrt(q) for this quarter
                            nc.scalar.activation(
                                out=q_q[:], in_=q_q[:], func=Act.Sqrt
                            )
                    q_tiles[g] = quarters

                def emit_g2(g, pso_tiles=pso_tiles, srow_ps=srow_ps,
                            q_tiles=q_tiles):
                    # w-pair at a time: two exps then immediately their
                    # GEMM2 + rowsum matmuls (keeps the PE HAM-warm)
                    quarters = q_tiles[g]
                    for jj in range(0, GRP, QB):
                        jb = g * GRP + jj
                        q_q = quarters[jj // QB]
                        w_b = w_pool.tile(
                            [P, QB, CHUNK], f8, tag="w", name="w_b"
                        )
                        if uniform_nit is not None:
                            # uniform temperature: constant scale, one
                            # activation per QB-block
                            nc.scalar.activation(
                                out=w_b[:],
                                in_=q_q[:],
                                func=Act.Exp,
                                scale=float(uniform_nit),
                            )
                        else:
                            for u in range(QB):
                                nc.scalar.activation(
                                    out=w_b[:, u, :],
                                    in_=q_q[:, u, :],
                                    func=Act.Exp,
                                    scale=nit_sb[:, jb + u : jb + u + 1],
                                )
                        for mp in range(0, QB, 2):
                            j = jb + mp
                            nc.tensor.matmul(
                                srow_ps,
                                ones2_sb,
                                w_b[:, mp : mp + 2, :],
                                perf_mode=DR,
                                start=(j == 0),
                                stop=(j == NT - 2),
                            )
                            for i in range(BTT):
                                nc.tensor.matmul(
                                    pso_tiles[i],
                                    w_b[:, mp : mp + 2, i * P : (i + 1) * P],
                                    v_sb[:, j : j + 2, :],
                                    perf_mode=DR,
                                    start=(j == 0),
                                    stop=(j == NT - 2),
                                )

                def make_drain(c0=c0, pso_tiles=pso_tiles, srow_ps=srow_ps):
                    def drain():
                        # normalize: out_i = pso_i * (1 / (64 (s_i + 1e-8)))
                        s_sb = sr_pool.tile([1, CHUNK], f32, tag="s", name="s_sb")
                        nc.vector.tensor_tensor(s_sb, srow_ps, eps_sb, Alu.add)
                        # transpose (s+eps) [1, CHUNK] -> [P, BTT] via K=1
                        # matmuls (single accumulation group; later MMs
                        # overwrite their own fresh columns)
                        rpt_ps = rpt_pool.tile([P, BTT], f32, tag="rpt", name="rpt")
                        for i in range(BTT):
                            nc.tensor.matmul(
                                rpt_ps[:, i : i + 1],
                                s_sb[0:1, i * P : (i + 1) * P],
                                one1_sb,
                                start=(i == 0),
                                stop=(i == BTT - 1),
                            )
                        r_sb = sr_pool.tile([P, BTT], f32, tag="r", name="r_sb")
                        nc.vector.reciprocal(out=r_sb, in_=rpt_ps)
                        for i in range(BTT):
                            o_sb = o_pool.tile([P, D], f32, tag="o", name="o_sb")
                            nc.vector.tensor_tensor(
                                o_sb,
                                pso_tiles[i],
                                r_sb[:, i : i + 1].to_broadcast([P, D]),
                                Alu.mult,
                            )
                            out_eng = nc.gpsimd if i % 2 == 0 else nc.sync
                            out_eng.dma_start(
                                out=out[c0 + i * P : c0 + (i + 1) * P, :],
                                in_=o_sb,
                            )
                    return drain

                for g in range(NG):
                    emit_g1(g)
                    if g == 0 and pending_drain[0] is not None:
                        # previous chunk's epilogue overlaps this GEMM1
                        pending_drain[0]()
                        pending_drain[0] = None
                for g in range(NG):
                    emit_g2(g)
                pending_drain[0] = make_drain()
            pending_drain[0]()
    nc.finalize()
    return nc


def prepare_in_maps(x, positions, values, temperature, n_cores=N_CORES):
    f8 = ml_dtypes.float8_e4m3
    x = np.asarray(x, np.float32)
    positions = np.asarray(positions, np.float32)
    values = np.asarray(values, np.float32)
    temperature = np.asarray(temperature, np.float32)

    B, T, D = x.shape
    N = positions.shape[0]
    xf = x.reshape(-1, D)
    R = xf.shape[0] // n_cores

    # attention scale with ages=0: 0.05 + 0.95 * (1 - exp(0)) = 0.05
    eff_t = (np.abs(temperature) + 0.1) * np.float32(0.05)
    nit_full = (-1.0 / eff_t).astype(np.float32)                 # [N]
    p2_full = (positions * positions).sum(1).astype(np.float32)  # [N]
    NT = N // P
    p2_pt = np.ascontiguousarray(p2_full.reshape(NT, P).T)
    nit_pt = np.ascontiguousarray(nit_full.reshape(NT, P).T)
    posTm2 = np.ascontiguousarray((-2.0 * positions).T).astype(f8)
    # *64 keeps fp8(v) away from the subnormal range; the kernel divides
    # it back out via the 64.0 in the row-sum transpose matmul
    v_f8 = np.ascontiguousarray(values * 64.0).astype(f8)

    maps = []
    for ci in range(n_cores):
        xc = xf[ci * R : (ci + 1) * R]
        x2c = (xc * xc).sum(1, dtype=np.float32)
        aux = np.empty((P, R + 2 * NT), np.float32)
        aux[:, :R] = x2c[None, :]
        aux[:, R : R + NT] = p2_pt
        aux[:, R + NT : R + 2 * NT] = nit_pt
        maps.append(
            dict(
                xT=np.ascontiguousarray(xc.T).astype(f8),
                posTm2=posTm2,
                v=v_f8,
                aux=aux,
            )
        )
    return maps


_prog_cache = {}


def get_program(uniform_nit=None):
    key = ("nc", uniform_nit)
    if key not in _prog_cache:
        _prog_cache[key] = build_program(uniform_nit=uniform_nit)
    return _prog_cache[key]


def kernel(x, positions, values, temperature):
    from concourse.bass_utils import run_bass_kernel_spmd

    temperature = np.asarray(temperature, np.float32)
    t0 = float(temperature.flat[0])
    uniform_nit = None
    if np.all(temperature == t0):
        uniform_nit = -1.0 / ((abs(t0) + 0.1) * 0.05)

    maps = prepare_in_maps(x, positions, values, temperature)
    nc = get_program(uniform_nit)
    res = run_bass_kernel_spmd(nc, maps, list(range(N_CORES)))
    B, T, D = np.asarray(x).shape
    out = np.concatenate(
        [np.asarray(res.results[i]["out"]) for i in range(N_CORES)], axis=0
    )
    return np.ascontiguousarray(out.reshape(B, T, D)).astype(np.float32)


# revision 38
# speedup vs baseline: 1.0216x; 1.0216x over previous
"""Trainium2 Bass kernel for nn_MultiScaleGeometricAttention.

Reference semantics (ages=0 => attention_scale = 0.05):
    eff_t[n] = (|temperature[n]| + 0.1) * 0.05
    q[r, n]  = ||x_r||^2 + ||p_n||^2 - 2 * (x_r . p_n)
    d = sqrt(q);   w = exp(-d / eff_t)
    out = (w @ values) / (w @ 1 + 1e-8)
(the per-row normalization commutes with the value GEMM, so it is applied
after both GEMMs)

Sharding: data-parallel over flattened B*T rows; 2048 rows per core on 8
cores; positions/values/temperature replicated.

Per-core device pipeline (layout S^T: n on partitions, rows on free axis):
    GEMM1 (PE, fp8 DoubleRow):  psq[n, r] = (-2 p)^T @ x   (f32 PSUM, K=512)
    DVE:   q = (psq + p2[n]) + x2[r]     (one scalar_tensor_tensor, bf16 out)
    ACT:   d = sqrt(q)                   (one batched activation per group)
    ACT:   w = exp(-d/t[n]) -> fp8       (per-j, per-partition scale)
    GEMM2 (PE, fp8 DoubleRow, paced per w-pair so the PE never sees a
           full HAM idle window):  o[r,:] += w_pair^T @ (64 v);
                                   s[r]  += ones^T @ w_pair
    PE:    transpose (s + eps) to partitions via K=1 matmuls (x64)
    DVE:   out = o * (1 / (64 (s + 1e-8)))
"""

import sys

if "/opt/trn_rl_repo" not in sys.path:
    sys.path.insert(0, "/opt/trn_rl_repo")

import numpy as np
import ml_dtypes

P = 128
CHUNK = 512  # row-columns of S^T processed per chunk (PSUM free dim)
GROUP = 32   # n-tiles per sqrt batch group

N_CORES = 8


def build_program(R=2048, N=4096, D=512, uniform_nit=None):
    import concourse.mybir as mybir
    import concourse.tile as tile
    from concourse import bacc

    f32 = mybir.dt.float32
    bf16 = mybir.dt.bfloat16
    f8 = mybir.dt.float8e4
    DR = mybir.MatmulPerfMode.DoubleRow
    Alu = mybir.AluOpType
    Act = mybir.ActivationFunctionType

    KT = D // P      # contraction tiles for GEMM1
    NT = N // P      # n tiles
    CH = R // CHUNK  # chunks
    BTT = CHUNK // P # row tiles per chunk
    GRP = min(GROUP, NT)
    NG = NT // GRP   # groups per chunk

    nc = bacc.Bacc()
    xT = nc.declare_dram_parameter("xT", [D, R], f8, isOutput=False)
    posTm2 = nc.declare_dram_parameter("posTm2", [D, N], f8, isOutput=False)
    v = nc.declare_dram_parameter("v", [N, D], f8, isOutput=False)
    # aux: [:, :R] = x2 broadcast, [:, R:R+NT] = p2, [:, R+NT:] = -1/eff_t
    aux = nc.declare_dram_parameter("aux", [P, R + 2 * NT], f32, isOutput=False)
    out = nc.declare_dram_parameter("out", [R, D], f32, isOutput=True)

    with tile.TileContext(nc) as tc:
        with (
            tc.tile_pool(name="singles", bufs=1) as singles,
            tc.tile_pool(name="xt", bufs=2) as xt_pool,
            tc.tile_pool(name="q", bufs=5) as q_pool,
            tc.tile_pool(name="w", bufs=3) as w_pool,
            tc.tile_pool(name="o", bufs=4) as o_pool,
            tc.tile_pool(name="sr", bufs=2) as sr_pool,
            tc.tile_pool(name="psq", bufs=2, space="PSUM") as psq_pool,
            tc.tile_pool(name="pso", bufs=1, space="PSUM") as pso_pool,
            tc.tile_pool(name="psr", bufs=1, space="PSUM") as psr_pool,
            tc.tile_pool(name="rpt", bufs=1, space="PSUM") as rpt_pool,
        ):
            xT_r = xT[:, :].rearrange("(kt p) r -> p kt r", p=P)
            xts = [None] * CH

            def load_xt(c):
                t = xt_pool.tile([P, KT, CHUNK], f8, tag="xt", name=f"xt{c}")
                nc.sync.dma_start(
                    out=t, in_=xT_r[:, :, c * CHUNK : (c + 1) * CHUNK]
                )
                xts[c] = t

            # first chunk of x lands first so GEMM1 can start ASAP
            load_xt(0)
            # positions as k-PAIR tiles (DoubleRow lhsT layout [Ki, 2, M])
            posTm2_r = posTm2[:, :].rearrange("(kp s p) n -> kp p s n", p=P, s=2)
            posT_tiles = []
            dma_engines = [nc.sync, nc.gpsimd]
            nsplit = max(P, N // 4)
            for kp in range(KT // 2):
                pt = singles.tile([P, 2, N], f8, name=f"posT{kp}")
                eng = dma_engines[kp % len(dma_engines)]
                # low-j columns land first so GEMM1 can start sooner
                eng.dma_start(out=pt[:, :, :nsplit], in_=posTm2_r[kp][:, :, :nsplit])
                posT_tiles.append(pt)
            # the DVE/ACT chain needs p2/nit + chunk-0 x2 right away: put
            # those small slices early on the sync ring
            aux_sb = singles.tile([P, R + 2 * NT], f32)
            nc.sync.dma_start(
                out=aux_sb[:, R : R + 2 * NT], in_=aux[:, R : R + 2 * NT]
            )
            nc.sync.dma_start(out=aux_sb[:, :CHUNK], in_=aux[:, :CHUNK])
            for kp in range(KT // 2):
                eng = dma_engines[kp % len(dma_engines)]
                eng.dma_start(
                    out=posT_tiles[kp][:, :, nsplit:],
                    in_=posTm2_r[kp][:, :, nsplit:],
                )
            if R > CHUNK:
                nc.gpsimd.dma_start(
                    out=aux_sb[:, CHUNK:R], in_=aux[:, CHUNK:R]
                )
            x2b_sb = aux_sb[:, :R]
            p2_sb = aux_sb[:, R : R + NT]
            nit_sb = aux_sb[:, R + NT : R + 2 * NT]
            v_sb = singles.tile([P, NT, D], f8)
            v_r = v[:, :].rearrange("(vh nt p) d -> vh p nt d", p=P, vh=4)
            v_engines = [nc.gpsimd, nc.gpsimd, nc.sync, nc.sync]
            for h in range(4):
                v_engines[h].dma_start(
                    out=v_sb[:, h * (NT // 4) : (h + 1) * (NT // 4), :], in_=v_r[h]
                )
            # padded so the DoubleRow K-pair stride is 16 (ISA constraint)
            ones2_sb_full = singles.tile([P, 2, 16], f8)
            nc.vector.memset(ones2_sb_full, 1.0)
            ones2_sb = ones2_sb_full[:, :, 0:1]
            # 64.0 compensates the host-side v*64 fp8 scaling:
            # rpt = 64*(s+eps); out = (64*w@v) / (64*(s+eps))
            one1_sb = singles.tile([1, 1], f32)
            nc.vector.memset(one1_sb, 64.0)
            eps_sb = singles.tile([1, CHUNK], f32)
            nc.vector.memset(eps_sb, 1e-8)

            pending_drain = [None]
            for c in range(CH):
                c0 = c * CHUNK
                if c + 1 < CH:
                    load_xt(c + 1)
                xt = xts[c]

                pso_tiles = [
                    pso_pool.tile([P, D], f32, tag=f"pso{i}", name=f"pso{i}")
                    for i in range(BTT)
                ]
                srow_ps = psr_pool.tile([1, CHUNK], f32, tag="psrow")
                q_tiles = [None] * NG

                QB = 8  # j's per sqrt/exp batch (block tiles)

                def emit_g1(g, xt=xt, c0=c0, q_tiles=q_tiles):
                    # GEMM1 (fp8 DR) + q assembly + quarter-batched sqrts
                    # (separate tiles -> fine-grained deps, the first exp
                    # can start after only QB columns of GEMM1)
                    quarters = []
                    for jj in range(GRP):
                        j = g * GRP + jj
                        if jj % QB == 0:
                            q_q = q_pool.tile(
                                [P, QB, CHUNK], bf16, tag="q", name="q_q"
                            )
                            quarters.append(q_q)
                        psq = psq_pool.tile([P, CHUNK], f32, tag="psq", name="psq")
                        for kp in range(KT // 2):
                            nc.tensor.matmul(
                                psq,
                                posT_tiles[kp][:, :, j * P : (j + 1) * P],
                                xt[:, 2 * kp : 2 * kp + 2, :],
                                perf_mode=DR,
                                start=(kp == 0),
                                stop=(kp == KT // 2 - 1),
                            )
                        # q = (psq + p2[n]) + x2[r]
                        nc.vector.scalar_tensor_tensor(
                            out=q_q[:, jj % QB, :],
                            in0=psq,
                            scalar=p2_sb[:, j : j + 1],
                            in1=x2b_sb[:, c0 : c0 + CHUNK],
                            op0=Alu.add,
                            op1=Alu.add,
                        )
                        if jj % QB == QB - 1:
                            # d = sqrt(q) for this quarter
                            nc.scalar.activation(
                                out=q_q[:], in_=q_q[:], func=Act.Sqrt
                            )
                    q_tiles[g] = quarters

                def emit_g2(g, pso_tiles=pso_tiles, srow_ps=srow_ps,
                            q_tiles=q_tiles):
                    # w-pair at a time: two exps then immediately their
                    # GEMM2 + rowsum matmuls (keeps the PE HAM-warm)
                    quarters = q_tiles[g]
                    for jj in range(0, GRP, QB):
                        jb = g * GRP + jj
                        q_q = quarters[jj // QB]
                        w_b = w_pool.tile(
                            [P, QB, CHUNK], f8, tag="w", name="w_b"
                        )
                        if uniform_nit is not None:
                            # uniform temperature: constant scale, one
                            # activation per QB-block
                            nc.scalar.activation(
                                out=w_b[:],
                                in_=q_q[:],
                                func=Act.Exp,
                                scale=float(uniform_nit),
                            )
                        else:
                            for u in range(QB):
                                nc.scalar.activation(
                                    out=w_b[:, u, :],
                                    in_=q_q[:, u, :],
                                    func=Act.Exp,
                                    scale=nit_sb[:, jb + u : jb + u + 1],
                                )
                        for mp in range(0, QB, 2):
                            j = jb + mp
                            nc.tensor.matmul(
                                srow_ps,
                                ones2_sb,
                                w_b[:, mp : mp + 2, :],
                                perf_mode=DR,
                                start=(j == 0),
                                stop=(j == NT - 2),
                            )
                            for i in range(BTT):
                                nc.tensor.matmul(
                                    pso_tiles[i],
                                    w_b[:, mp : mp + 2, i * P : (i + 1) * P],
                                    v_sb[:, j : j + 2, :],
                                    perf_mode=DR,
                                    start=(j == 0),
                                    stop=(j == NT - 2),
                                )

                def make_drain(c0=c0, pso_tiles=pso_tiles, srow_ps=srow_ps):
                    def drain():
                        # normalize: out_i = pso_i * (1 / (64 (s_i + 1e-8)))
                        s_sb = sr_pool.tile([1, CHUNK], f32, tag="s", name="s_sb")
                        nc.vector.tensor_tensor(s_sb, srow_ps, eps_sb, Alu.add)
                        # transpose (s+eps) [1, CHUNK] -> [P, BTT] via K=1
                        # matmuls (single accumulation group; later MMs
                        # overwrite their own fresh columns)
                        rpt_ps = rpt_pool.tile([P, BTT], f32, tag="rpt", name="rpt")
                        for i in range(BTT):
                            nc.tensor.matmul(
                                rpt_ps[:, i : i + 1],
                                s_sb[0:1, i * P : (i + 1) * P],
                                one1_sb,
                                start=(i == 0),
                                stop=(i == BTT - 1),
                            )
                        r_sb = sr_pool.tile([P, BTT], f32, tag="r", name="r_sb")
                        nc.vector.reciprocal(out=r_sb, in_=rpt_ps)
                        for i in range(BTT):
                            o_sb = o_pool.tile([P, D], f32, tag="o", name="o_sb")
                            nc.vector.tensor_tensor(
                                o_sb,
                                pso_tiles[i],
                                r_sb[:, i : i + 1].to_broadcast([P, D]),
                                Alu.mult,
                            )
                            out_eng = nc.gpsimd if i % 2 == 0 else nc.sync
                            out_eng.dma_start(
                                out=out[c0 + i * P : c0 + (i + 1) * P, :],
                                in_=o_sb,
                            )
                    return drain

                for g in range(NG):
                    emit_g1(g)
                    if g == 0 and pending_drain[0] is not None:
                        # previous chunk's epilogue overlaps this GEMM1
                        pending_drain[0]()
                        pending_drain[0] = None
                for g in range(NG):
                    emit_g2(g)
                pending_drain[0] = make_drain()
            pending_drain[0]()
    nc.finalize()
    return nc


def prepare_in_maps(x, positions, values, temperature, n_cores=N_CORES):
    f8 = ml_dtypes.float8_e4m3
    x = np.asarray(x, np.float32)
    positions = np.asarray(positions, np.float32)
    values = np.asarray(values, np.float32)
    temperature = np.asarray(temperature, np.float32)

    B, T, D = x.shape
    N = positions.shape[0]
    xf = x.reshape(-1, D)
    R = xf.shape[0] // n_cores

    # attention scale with ages=0: 0.05 + 0.95 * (1 - exp(0)) = 0.05
    eff_t = (np.abs(temperature) + 0.1) * np.float32(0.05)
    nit_full = (-1.0 / eff_t).astype(np.float32)                 # [N]
    p2_full = (positions * positions).sum(1).astype(np.float32)  # [N]
    NT = N // P
    p2_pt = np.ascontiguousarray(p2_full.reshape(NT, P).T)
    nit_pt = np.ascontiguousarray(nit_full.reshape(NT, P).T)
    posTm2 = np.ascontiguousarray((-2.0 * positions).T).astype(f8)
    # *64 keeps fp8(v) away from the subnormal range; the kernel divides
    # it back out via the 64.0 in the row-sum transpose matmul
    v_f8 = np.ascontiguousarray(values * 64.0).astype(f8)

    maps = []
    for ci in range(n_cores):
        xc = xf[ci * R : (ci + 1) * R]
        x2c = (xc * xc).sum(1, dtype=np.float32)
        aux = np.empty((P, R + 2 * NT), np.float32)
        aux[:, :R] = x2c[None, :]
        aux[:, R : R + NT] = p2_pt
        aux[:, R + NT : R + 2 * NT] = nit_pt
        maps.append(
            dict(
                xT=np.ascontiguousarray(xc.T).astype(f8),
                posTm2=posTm2,
                v=v_f8,
                aux=aux,
            )
        )
    return maps


_prog_cache = {}


def get_program(uniform_nit=None):
    key = ("nc", uniform_nit)
    if key not in _prog_cache:
        _prog_cache[key] = build_program(uniform_nit=uniform_nit)
    return _prog_cache[key]


def kernel(x, positions, values, temperature):
    from concourse.bass_utils import run_bass_kernel_spmd

    temperature = np.asarray(temperature, np.float32)
    t0 = float(temperature.flat[0])
    uniform_nit = None
    if np.all(temperature == t0):
        uniform_nit = -1.0 / ((abs(t0) + 0.1) * 0.05)

    maps = prepare_in_maps(x, positions, values, temperature)
    nc = get_program(uniform_nit)
    res = run_bass_kernel_spmd(nc, maps, list(range(N_CORES)))
    B, T, D = np.asarray(x).shape
    out = np.concatenate(
        [np.asarray(res.results[i]["out"]) for i in range(N_CORES)], axis=0
    )
    return np.ascontiguousarray(out.reshape(B, T, D)).astype(np.float32)


# revision 39
# speedup vs baseline: 1.0913x; 1.0683x over previous
"""Trainium2 Bass kernel for nn_MultiScaleGeometricAttention.

Reference semantics (ages=0 => attention_scale = 0.05):
    eff_t[n] = (|temperature[n]| + 0.1) * 0.05
    q[r, n]  = ||x_r||^2 + ||p_n||^2 - 2 * (x_r . p_n)
    d = sqrt(q);   w = exp(-d / eff_t)
    out = (w @ values) / (w @ 1 + 1e-8)
(the per-row normalization commutes with the value GEMM, so it is applied
after both GEMMs)

Sharding: data-parallel over flattened B*T rows; 2048 rows per core on 8
cores; positions/values/temperature replicated.

Per-core device pipeline (layout S^T: n on partitions, rows on free axis):
    GEMM1 (PE, fp8 DoubleRow):  psq[n, r] = (-2 p)^T @ x   (f32 PSUM, K=512)
    DVE:   q = (psq + p2[n]) + x2[r]     (one scalar_tensor_tensor, bf16 out)
    ACT:   d = sqrt(q)                   (one batched activation per group)
    ACT:   w = exp(-d/t[n]) -> fp8       (per-j, per-partition scale)
    GEMM2 (PE, fp8 DoubleRow, paced per w-pair so the PE never sees a
           full HAM idle window):  o[r,:] += w_pair^T @ (64 v);
                                   s[r]  += ones^T @ w_pair
    PE:    transpose (s + eps) to partitions via K=1 matmuls (x64)
    DVE:   out = o * (1 / (64 (s + 1e-8)))
"""

import sys

if "/opt/trn_rl_repo" not in sys.path:
    sys.path.insert(0, "/opt/trn_rl_repo")

import numpy as np
import ml_dtypes

P = 128
CHUNK = 512  # row-columns of S^T processed per chunk (PSUM free dim)
GROUP = 32   # n-tiles per sqrt batch group

N_CORES = 8


def build_program(R=2048, N=4096, D=512, uniform_nit=None):
    import concourse.mybir as mybir
    import concourse.tile as tile
    from concourse import bacc

    f32 = mybir.dt.float32
    bf16 = mybir.dt.bfloat16
    f8 = mybir.dt.float8e4
    DR = mybir.MatmulPerfMode.DoubleRow
    Alu = mybir.AluOpType
    Act = mybir.ActivationFunctionType

    KT = D // P      # contraction tiles for GEMM1
    NT = N // P      # n tiles
    CH = R // CHUNK  # chunks
    BTT = CHUNK // P # row tiles per chunk
    GRP = min(GROUP, NT)
    NG = NT // GRP   # groups per chunk

    nc = bacc.Bacc()
    xT = nc.declare_dram_parameter("xT", [D, R], f8, isOutput=False)
    posTm2 = nc.declare_dram_parameter("posTm2", [D, N], f8, isOutput=False)
    v = nc.declare_dram_parameter("v", [N, D], f8, isOutput=False)
    # aux: [:, :R] = x2 broadcast, [:, R:R+NT] = p2, [:, R+NT:] = -1/eff_t
    aux = nc.declare_dram_parameter("aux", [P, R + 2 * NT], f32, isOutput=False)
    out = nc.declare_dram_parameter("out", [R, D], f32, isOutput=True)

    with tile.TileContext(nc) as tc:
        with (
            tc.tile_pool(name="singles", bufs=1) as singles,
            tc.tile_pool(name="xt", bufs=2) as xt_pool,
            tc.tile_pool(name="q", bufs=10) as q_pool,
            tc.tile_pool(name="w", bufs=6) as w_pool,
            tc.tile_pool(name="o", bufs=4) as o_pool,
            tc.tile_pool(name="sr", bufs=2) as sr_pool,
            tc.tile_pool(name="psq", bufs=2, space="PSUM") as psq_pool,
            tc.tile_pool(name="pso", bufs=1, space="PSUM") as pso_pool,
            tc.tile_pool(name="psr", bufs=1, space="PSUM") as psr_pool,
            tc.tile_pool(name="rpt", bufs=1, space="PSUM") as rpt_pool,
        ):
            xT_r = xT[:, :].rearrange("(kt p) r -> p kt r", p=P)
            xts = [None] * CH

            def load_xt(c):
                t = xt_pool.tile([P, KT, CHUNK], f8, tag="xt", name=f"xt{c}")
                nc.sync.dma_start(
                    out=t, in_=xT_r[:, :, c * CHUNK : (c + 1) * CHUNK]
                )
                xts[c] = t

            # first chunk of x lands first so GEMM1 can start ASAP
            load_xt(0)
            # positions as k-PAIR tiles (DoubleRow lhsT layout [Ki, 2, M])
            posTm2_r = posTm2[:, :].rearrange("(kp s p) n -> kp p s n", p=P, s=2)
            posT_tiles = []
            dma_engines = [nc.sync, nc.gpsimd]
            nsplit = max(P, N // 4)
            for kp in range(KT // 2):
                pt = singles.tile([P, 2, N], f8, name=f"posT{kp}")
                eng = dma_engines[kp % len(dma_engines)]
                # low-j columns land first so GEMM1 can start sooner
                eng.dma_start(out=pt[:, :, :nsplit], in_=posTm2_r[kp][:, :, :nsplit])
                posT_tiles.append(pt)
            # the DVE/ACT chain needs p2/nit + chunk-0 x2 right away: put
            # those small slices early on the sync ring
            aux_sb = singles.tile([P, R + 2 * NT], f32)
            nc.sync.dma_start(
                out=aux_sb[:, R : R + 2 * NT], in_=aux[:, R : R + 2 * NT]
            )
            nc.sync.dma_start(out=aux_sb[:, :CHUNK], in_=aux[:, :CHUNK])
            for kp in range(KT // 2):
                eng = dma_engines[kp % len(dma_engines)]
                eng.dma_start(
                    out=posT_tiles[kp][:, :, nsplit:],
                    in_=posTm2_r[kp][:, :, nsplit:],
                )
            if R > CHUNK:
                nc.gpsimd.dma_start(
                    out=aux_sb[:, CHUNK:R], in_=aux[:, CHUNK:R]
                )
            x2b_sb = aux_sb[:, :R]
            p2_sb = aux_sb[:, R : R + NT]
            nit_sb = aux_sb[:, R + NT : R + 2 * NT]
            v_sb = singles.tile([P, NT, D], f8)
            v_r = v[:, :].rearrange("(vh nt p) d -> vh p nt d", p=P, vh=4)
            v_engines = [nc.gpsimd, nc.gpsimd, nc.sync, nc.sync]
            for h in range(4):
                v_engines[h].dma_start(
                    out=v_sb[:, h * (NT // 4) : (h + 1) * (NT // 4), :], in_=v_r[h]
                )
            # padded so the DoubleRow K-pair stride is 16 (ISA constraint)
            ones2_sb_full = singles.tile([P, 2, 16], f8)
            nc.vector.memset(ones2_sb_full, 1.0)
            ones2_sb = ones2_sb_full[:, :, 0:1]
            # 64.0 compensates the host-side v*64 fp8 scaling:
            # rpt = 64*(s+eps); out = (64*w@v) / (64*(s+eps))
            one1_sb = singles.tile([1, 1], f32)
            nc.vector.memset(one1_sb, 64.0)
            eps_sb = singles.tile([1, CHUNK], f32)
            nc.vector.memset(eps_sb, 1e-8)

            pending_drain = [None]
            for c in range(CH):
                c0 = c * CHUNK
                if c + 1 < CH:
                    load_xt(c + 1)
                xt = xts[c]

                pso_tiles = [
                    pso_pool.tile([P, D], f32, tag=f"pso{i}", name=f"pso{i}")
                    for i in range(BTT)
                ]
                srow_ps = psr_pool.tile([1, CHUNK], f32, tag="psrow")
                q_tiles = [None] * NG

                QB = 4  # j's per sqrt batch (quarter-group tiles)

                def emit_g1(g, xt=xt, c0=c0, q_tiles=q_tiles):
                    # GEMM1 (fp8 DR) + q assembly + quarter-batched sqrts
                    # (separate tiles -> fine-grained deps, the first exp
                    # can start after only QB columns of GEMM1)
                    quarters = []
                    for jj in range(GRP):
                        j = g * GRP + jj
                        if jj % QB == 0:
                            q_q = q_pool.tile(
                                [P, QB, CHUNK], bf16, tag="q", name="q_q"
                            )
                            quarters.append(q_q)
                        psq = psq_pool.tile([P, CHUNK], f32, tag="psq", name="psq")
                        for kp in range(KT // 2):
                            nc.tensor.matmul(
                                psq,
                                posT_tiles[kp][:, :, j * P : (j + 1) * P],
                                xt[:, 2 * kp : 2 * kp + 2, :],
                                perf_mode=DR,
                                start=(kp == 0),
                                stop=(kp == KT // 2 - 1),
                            )
                        # q = (psq + p2[n]) + x2[r]
                        nc.vector.scalar_tensor_tensor(
                            out=q_q[:, jj % QB, :],
                            in0=psq,
                            scalar=p2_sb[:, j : j + 1],
                            in1=x2b_sb[:, c0 : c0 + CHUNK],
                            op0=Alu.add,
                            op1=Alu.add,
                        )
                        if jj % QB == QB - 1:
                            # d = sqrt(q) for this quarter
                            nc.scalar.activation(
                                out=q_q[:], in_=q_q[:], func=Act.Sqrt
                            )
                    q_tiles[g] = quarters

                def emit_g2(g, pso_tiles=pso_tiles, srow_ps=srow_ps,
                            q_tiles=q_tiles):
                    # w-pair at a time: two exps then immediately their
                    # GEMM2 + rowsum matmuls (keeps the PE HAM-warm)
                    quarters = q_tiles[g]
                    for jj in range(0, GRP, 2):
                        j = g * GRP + jj
                        q_q = quarters[jj // QB]
                        w_pair = w_pool.tile(
                            [P, 2, CHUNK], f8, tag="w", name="w_pair"
                        )
                        if uniform_nit is not None:
                            # uniform temperature: constant scale, one
                            # activation per pair (half the instr overhead)
                            nc.scalar.activation(
                                out=w_pair[:],
                                in_=q_q[:, (jj % QB) : (jj % QB) + 2, :],
                                func=Act.Exp,
                                scale=float(uniform_nit),
                            )
                        else:
                            for u in range(2):
                                nc.scalar.activation(
                                    out=w_pair[:, u, :],
                                    in_=q_q[:, (jj % QB) + u, :],
                                    func=Act.Exp,
                                    scale=nit_sb[:, j + u : j + u + 1],
                                )
                        nc.tensor.matmul(
                            srow_ps,
                            ones2_sb,
                            w_pair,
                            perf_mode=DR,
                            start=(j == 0),
                            stop=(j == NT - 2),
                        )
                        for i in range(BTT):
                            nc.tensor.matmul(
                                pso_tiles[i],
                                w_pair[:, :, i * P : (i + 1) * P],
                                v_sb[:, j : j + 2, :],
                                perf_mode=DR,
                                start=(j == 0),
                                stop=(j == NT - 2),
                            )

                def make_drain(c0=c0, pso_tiles=pso_tiles, srow_ps=srow_ps):
                    def drain():
                        # normalize: out_i = pso_i * (1 / (64 (s_i + 1e-8)))
                        s_sb = sr_pool.tile([1, CHUNK], f32, tag="s", name="s_sb")
                        nc.vector.tensor_tensor(s_sb, srow_ps, eps_sb, Alu.add)
                        # transpose (s+eps) [1, CHUNK] -> [P, BTT] via K=1
                        # matmuls (single accumulation group; later MMs
                        # overwrite their own fresh columns)
                        rpt_ps = rpt_pool.tile([P, BTT], f32, tag="rpt", name="rpt")
                        for i in range(BTT):
                            nc.tensor.matmul(
                                rpt_ps[:, i : i + 1],
                                s_sb[0:1, i * P : (i + 1) * P],
                                one1_sb,
                                start=(i == 0),
                                stop=(i == BTT - 1),
                            )
                        r_sb = sr_pool.tile([P, BTT], f32, tag="r", name="r_sb")
                        nc.vector.reciprocal(out=r_sb, in_=rpt_ps)
                        for i in range(BTT):
                            o_sb = o_pool.tile([P, D], f32, tag="o", name="o_sb")
                            nc.vector.tensor_tensor(
                                o_sb,
                                pso_tiles[i],
                                r_sb[:, i : i + 1].to_broadcast([P, D]),
                                Alu.mult,
                            )
                            out_eng = nc.gpsimd if i % 2 == 0 else nc.sync
                            out_eng.dma_start(
                                out=out[c0 + i * P : c0 + (i + 1) * P, :],
                                in_=o_sb,
                            )
                    return drain

                for g in range(NG):
                    emit_g1(g)
                    if g == 0 and pending_drain[0] is not None:
                        # previous chunk's epilogue overlaps this GEMM1
                        pending_drain[0]()
                        pending_drain[0] = None
                for g in range(NG):
                    emit_g2(g)
                pending_drain[0] = make_drain()
            pending_drain[0]()
    nc.finalize()
    return nc


def prepare_in_maps(x, positions, values, temperature, n_cores=N_CORES):
    f8 = ml_dtypes.float8_e4m3
    x = np.asarray(x, np.float32)
    positions = np.asarray(positions, np.float32)
    values = np.asarray(values, np.float32)
    temperature = np.asarray(temperature, np.float32)

    B, T, D = x.shape
    N = positions.shape[0]
    xf = x.reshape(-1, D)
    R = xf.shape[0] // n_cores

    # attention scale with ages=0: 0.05 + 0.95 * (1 - exp(0)) = 0.05
    eff_t = (np.abs(temperature) + 0.1) * np.float32(0.05)
    nit_full = (-1.0 / eff_t).astype(np.float32)                 # [N]
    p2_full = (positions * positions).sum(1).astype(np.float32)  # [N]
    NT = N // P
    p2_pt = np.ascontiguousarray(p2_full.reshape(NT, P).T)
    nit_pt = np.ascontiguousarray(nit_full.reshape(NT, P).T)
    posTm2 = np.ascontiguousarray((-2.0 * positions).T).astype(f8)
    # *64 keeps fp8(v) away from the subnormal range; the kernel divides
    # it back out via the 64.0 in the row-sum transpose matmul
    v_f8 = np.ascontiguousarray(values * 64.0).astype(f8)

    maps = []
    for ci in range(n_cores):
        xc = xf[ci * R : (ci + 1) * R]
        x2c = (xc * xc).sum(1, dtype=np.float32)
        aux = np.empty((P, R + 2 * NT), np.float32)
        aux[:, :R] = x2c[None, :]
        aux[:, R : R + NT] = p2_pt
        aux[:, R + NT : R + 2 * NT] = nit_pt
        maps.append(
            dict(
                xT=np.ascontiguousarray(xc.T).astype(f8),
                posTm2=posTm2,
                v=v_f8,
                aux=aux,
            )
        )
    return maps


_prog_cache = {}


def get_program(uniform_nit=None):
    key = ("nc", uniform_nit)
    if key not in _prog_cache:
        _prog_cache[key] = build_program(uniform_nit=uniform_nit)
    return _prog_cache[key]


def kernel(x, positions, values, temperature):
    from concourse.bass_utils import run_bass_kernel_spmd

    temperature = np.asarray(temperature, np.float32)
    t0 = float(temperature.flat[0])
    uniform_nit = None
    if np.all(temperature == t0):
        uniform_nit = -1.0 / ((abs(t0) + 0.1) * 0.05)

    maps = prepare_in_maps(x, positions, values, temperature)
    nc = get_program(uniform_nit)
    res = run_bass_kernel_spmd(nc, maps, list(range(N_CORES)))
    B, T, D = np.asarray(x).shape
    out = np.concatenate(
        [np.asarray(res.results[i]["out"]) for i in range(N_CORES)], axis=0
    )
    return np.ascontiguousarray(out.reshape(B, T, D)).astype(np.float32)


# revision 40
# speedup vs baseline: 1.1158x; 1.0224x over previous
"""Trainium2 Bass kernel for nn_MultiScaleGeometricAttention.

Reference semantics (ages=0 => attention_scale = 0.05):
    eff_t[n] = (|temperature[n]| + 0.1) * 0.05
    q[r, n]  = ||x_r||^2 + ||p_n||^2 - 2 * (x_r . p_n)
    d = sqrt(q);   w = exp(-d / eff_t)
    out = (w @ values) / (w @ 1 + 1e-8)
(the per-row normalization commutes with the value GEMM, so it is applied
after both GEMMs)

Sharding: data-parallel over flattened B*T rows; 2048 rows per core on 8
cores; positions/values/temperature replicated.

Per-core device pipeline (layout S^T: n on partitions, rows on free axis):
    GEMM1 (PE, fp8 DoubleRow):  psq[n, r] = (-2 p)^T @ x   (f32 PSUM, K=512)
    DVE:   q = (psq + p2[n]) + x2[r]     (one scalar_tensor_tensor, bf16 out)
    ACT:   d = sqrt(q)                   (one batched activation per group)
    ACT:   w = exp(-d/t[n]) -> fp8       (per-j, per-partition scale)
    GEMM2 (PE, fp8 DoubleRow, paced per w-pair so the PE never sees a
           full HAM idle window):  o[r,:] += w_pair^T @ (64 v);
                                   s[r]  += ones^T @ w_pair
    PE:    transpose (s + eps) to partitions via K=1 matmuls (x64)
    DVE:   out = o * (1 / (64 (s + 1e-8)))
"""

import sys

if "/opt/trn_rl_repo" not in sys.path:
    sys.path.insert(0, "/opt/trn_rl_repo")

import numpy as np
import ml_dtypes

P = 128
CHUNK = 512  # row-columns of S^T processed per chunk (PSUM free dim)
GROUP = 32   # n-tiles per sqrt batch group

N_CORES = 8


def build_program(R=2048, N=4096, D=512, uniform_nit=None):
    import concourse.mybir as mybir
    import concourse.tile as tile
    from concourse import bacc

    f32 = mybir.dt.float32
    bf16 = mybir.dt.bfloat16
    f8 = mybir.dt.float8e4
    DR = mybir.MatmulPerfMode.DoubleRow
    Alu = mybir.AluOpType
    Act = mybir.ActivationFunctionType

    KT = D // P      # contraction tiles for GEMM1
    NT = N // P      # n tiles
    CH = R // CHUNK  # chunks
    BTT = CHUNK // P # row tiles per chunk
    GRP = min(GROUP, NT)
    NG = NT // GRP   # groups per chunk

    nc = bacc.Bacc()
    xT = nc.declare_dram_parameter("xT", [D, R], f8, isOutput=False)
    posTm2 = nc.declare_dram_parameter("posTm2", [D, N], f8, isOutput=False)
    v = nc.declare_dram_parameter("v", [N, D], f8, isOutput=False)
    # aux: [:, :R] = x2 broadcast, [:, R:R+NT] = p2, [:, R+NT:] = -1/eff_t
    aux = nc.declare_dram_parameter("aux", [P, R + 2 * NT], f32, isOutput=False)
    out = nc.declare_dram_parameter("out", [R, D], f32, isOutput=True)

    with tile.TileContext(nc) as tc:
        with (
            tc.tile_pool(name="singles", bufs=1) as singles,
            tc.tile_pool(name="xt", bufs=2) as xt_pool,
            tc.tile_pool(name="q", bufs=10) as q_pool,
            tc.tile_pool(name="w", bufs=6) as w_pool,
            tc.tile_pool(name="o", bufs=4) as o_pool,
            tc.tile_pool(name="sr", bufs=2) as sr_pool,
            tc.tile_pool(name="psq", bufs=2, space="PSUM") as psq_pool,
            tc.tile_pool(name="pso", bufs=1, space="PSUM") as pso_pool,
            tc.tile_pool(name="psr", bufs=1, space="PSUM") as psr_pool,
            tc.tile_pool(name="rpt", bufs=1, space="PSUM") as rpt_pool,
        ):
            xT_r = xT[:, :].rearrange("(kt p) r -> p kt r", p=P)
            xts = [None] * CH

            def load_xt(c):
                t = xt_pool.tile([P, KT, CHUNK], f8, tag="xt", name=f"xt{c}")
                nc.sync.dma_start(
                    out=t, in_=xT_r[:, :, c * CHUNK : (c + 1) * CHUNK]
                )
                xts[c] = t

            # first chunk of x lands first so GEMM1 can start ASAP
            load_xt(0)
            # positions as k-PAIR tiles (DoubleRow lhsT layout [Ki, 2, M])
            posTm2_r = posTm2[:, :].rearrange("(kp s p) n -> kp p s n", p=P, s=2)
            posT_tiles = []
            dma_engines = [nc.sync, nc.gpsimd]
            nsplit = max(P, N // 4)
            for kp in range(KT // 2):
                pt = singles.tile([P, 2, N], f8, name=f"posT{kp}")
                eng = dma_engines[kp % len(dma_engines)]
                # low-j columns land first so GEMM1 can start sooner
                eng.dma_start(out=pt[:, :, :nsplit], in_=posTm2_r[kp][:, :, :nsplit])
                posT_tiles.append(pt)
            # the DVE/ACT chain needs p2/nit + chunk-0 x2 right away: use
            # the otherwise-idle Activation HWDGE ring (its triggers run
            # long before the first ACTIVATE)
            aux_sb = singles.tile([P, R + 2 * NT], f32)
            nc.scalar.dma_start(
                out=aux_sb[:, R : R + 2 * NT], in_=aux[:, R : R + 2 * NT]
            )
            nc.scalar.dma_start(out=aux_sb[:, :CHUNK], in_=aux[:, :CHUNK])
            for kp in range(KT // 2):
                eng = dma_engines[kp % len(dma_engines)]
                eng.dma_start(
                    out=posT_tiles[kp][:, :, nsplit:],
                    in_=posTm2_r[kp][:, :, nsplit:],
                )
            if R > CHUNK:
                nc.gpsimd.dma_start(
                    out=aux_sb[:, CHUNK:R], in_=aux[:, CHUNK:R]
                )
            x2b_sb = aux_sb[:, :R]
            p2_sb = aux_sb[:, R : R + NT]
            nit_sb = aux_sb[:, R + NT : R + 2 * NT]
            v_sb = singles.tile([P, NT, D], f8)
            v_r = v[:, :].rearrange("(vh nt p) d -> vh p nt d", p=P, vh=4)
            v_engines = [nc.gpsimd, nc.gpsimd, nc.sync, nc.sync]
            for h in range(4):
                v_engines[h].dma_start(
                    out=v_sb[:, h * (NT // 4) : (h + 1) * (NT // 4), :], in_=v_r[h]
                )
            # padded so the DoubleRow K-pair stride is 16 (ISA constraint)
            ones2_sb_full = singles.tile([P, 2, 16], f8)
            nc.vector.memset(ones2_sb_full, 1.0)
            ones2_sb = ones2_sb_full[:, :, 0:1]
            # 64.0 compensates the host-side v*64 fp8 scaling:
            # rpt = 64*(s+eps); out = (64*w@v) / (64*(s+eps))
            one1_sb = singles.tile([1, 1], f32)
            nc.vector.memset(one1_sb, 64.0)
            eps_sb = singles.tile([1, CHUNK], f32)
            nc.vector.memset(eps_sb, 1e-8)

            pending_drain = [None]
            for c in range(CH):
                c0 = c * CHUNK
                if c + 1 < CH:
                    load_xt(c + 1)
                xt = xts[c]

                pso_tiles = [
                    pso_pool.tile([P, D], f32, tag=f"pso{i}", name=f"pso{i}")
                    for i in range(BTT)
                ]
                srow_ps = psr_pool.tile([1, CHUNK], f32, tag="psrow")
                q_tiles = [None] * NG

                QB = 4  # j's per sqrt batch (quarter-group tiles)

                def emit_g1(g, xt=xt, c0=c0, q_tiles=q_tiles):
                    # GEMM1 (fp8 DR) + q assembly + quarter-batched sqrts
                    # (separate tiles -> fine-grained deps, the first exp
                    # can start after only QB columns of GEMM1)
                    quarters = []
                    for jj in range(GRP):
                        j = g * GRP + jj
                        if jj % QB == 0:
                            q_q = q_pool.tile(
                                [P, QB, CHUNK], bf16, tag="q", name="q_q"
                            )
                            quarters.append(q_q)
                        psq = psq_pool.tile([P, CHUNK], f32, tag="psq", name="psq")
                        for kp in range(KT // 2):
                            nc.tensor.matmul(
                                psq,
                                posT_tiles[kp][:, :, j * P : (j + 1) * P],
                                xt[:, 2 * kp : 2 * kp + 2, :],
                                perf_mode=DR,
                                start=(kp == 0),
                                stop=(kp == KT // 2 - 1),
                            )
                        # q = (psq + p2[n]) + x2[r]
                        nc.vector.scalar_tensor_tensor(
                            out=q_q[:, jj % QB, :],
                            in0=psq,
                            scalar=p2_sb[:, j : j + 1],
                            in1=x2b_sb[:, c0 : c0 + CHUNK],
                            op0=Alu.add,
                            op1=Alu.add,
                        )
                        if jj % QB == QB - 1:
                            # d = sqrt(q) for this quarter
                            nc.scalar.activation(
                                out=q_q[:], in_=q_q[:], func=Act.Sqrt
                            )
                    q_tiles[g] = quarters

                def emit_g2(g, pso_tiles=pso_tiles, srow_ps=srow_ps,
                            q_tiles=q_tiles):
                    # w-pair at a time: two exps then immediately their
                    # GEMM2 + rowsum matmuls (keeps the PE HAM-warm)
                    quarters = q_tiles[g]
                    for jj in range(0, GRP, 2):
                        j = g * GRP + jj
                        q_q = quarters[jj // QB]
                        w_pair = w_pool.tile(
                            [P, 2, CHUNK], f8, tag="w", name="w_pair"
                        )
                        if uniform_nit is not None:
                            # uniform temperature: constant scale, one
                            # activation per pair (half the instr overhead)
                            nc.scalar.activation(
                                out=w_pair[:],
                                in_=q_q[:, (jj % QB) : (jj % QB) + 2, :],
                                func=Act.Exp,
                                scale=float(uniform_nit),
                            )
                        else:
                            for u in range(2):
                                nc.scalar.activation(
                                    out=w_pair[:, u, :],
                                    in_=q_q[:, (jj % QB) + u, :],
                                    func=Act.Exp,
                                    scale=nit_sb[:, j + u : j + u + 1],
                                )
                        nc.tensor.matmul(
                            srow_ps,
                            ones2_sb,
                            w_pair,
                            perf_mode=DR,
                            start=(j == 0),
                            stop=(j == NT - 2),
                        )
                        for i in range(BTT):
                            nc.tensor.matmul(
                                pso_tiles[i],
                                w_pair[:, :, i * P : (i + 1) * P],
                                v_sb[:, j : j + 2, :],
                                perf_mode=DR,
                                start=(j == 0),
                                stop=(j == NT - 2),
                            )

                def make_drain(c0=c0, pso_tiles=pso_tiles, srow_ps=srow_ps,
                               last=(c == CH - 1)):
                    def drain():
                        # normalize: out_i = pso_i * (1 / (64 (s_i + 1e-8)))
                        s_sb = sr_pool.tile([1, CHUNK], f32, tag="s", name="s_sb")
                        nc.vector.tensor_tensor(s_sb, srow_ps, eps_sb, Alu.add)
                        # transpose (s+eps) [1, CHUNK] -> [P, BTT] via K=1
                        # matmuls (single accumulation group; later MMs
                        # overwrite their own fresh columns)
                        rpt_ps = rpt_pool.tile([P, BTT], f32, tag="rpt", name="rpt")
                        for i in range(BTT):
                            nc.tensor.matmul(
                                rpt_ps[:, i : i + 1],
                                s_sb[0:1, i * P : (i + 1) * P],
                                one1_sb,
                                start=(i == 0),
                                stop=(i == BTT - 1),
                            )
                        r_sb = sr_pool.tile([P, BTT], f32, tag="r", name="r_sb")
                        nc.vector.reciprocal(out=r_sb, in_=rpt_ps)
                        for i in range(BTT):
                            o_sb = o_pool.tile([P, D], f32, tag="o", name="o_sb")
                            nc.vector.tensor_tensor(
                                o_sb,
                                pso_tiles[i],
                                r_sb[:, i : i + 1].to_broadcast([P, D]),
                                Alu.mult,
                            )
                            if last:
                                engs = [nc.gpsimd, nc.sync, nc.scalar, nc.scalar]
                            else:
                                engs = [nc.gpsimd, nc.sync, nc.gpsimd, nc.sync]
                            out_eng = engs[i % 4]
                            out_eng.dma_start(
                                out=out[c0 + i * P : c0 + (i + 1) * P, :],
                                in_=o_sb,
                            )
                    return drain

                for g in range(NG):
                    emit_g1(g)
                    if g == 0 and pending_drain[0] is not None:
                        # previous chunk's epilogue overlaps this GEMM1
                        pending_drain[0]()
                        pending_drain[0] = None
                for g in range(NG):
                    emit_g2(g)
                pending_drain[0] = make_drain()
            pending_drain[0]()
    nc.finalize()
    return nc


def prepare_in_maps(x, positions, values, temperature, n_cores=N_CORES):
    f8 = ml_dtypes.float8_e4m3
    x = np.asarray(x, np.float32)
    positions = np.asarray(positions, np.float32)
    values = np.asarray(values, np.float32)
    temperature = np.asarray(temperature, np.float32)

    B, T, D = x.shape
    N = positions.shape[0]
    xf = x.reshape(-1, D)
    R = xf.shape[0] // n_cores

    # attention scale with ages=0: 0.05 + 0.95 * (1 - exp(0)) = 0.05
    eff_t = (np.abs(temperature) + 0.1) * np.float32(0.05)
    nit_full = (-1.0 / eff_t).astype(np.float32)                 # [N]
    p2_full = (positions * positions).sum(1).astype(np.float32)  # [N]
    NT = N // P
    p2_pt = np.ascontiguousarray(p2_full.reshape(NT, P).T)
    nit_pt = np.ascontiguousarray(nit_full.reshape(NT, P).T)
    posTm2 = np.ascontiguousarray((-2.0 * positions).T).astype(f8)
    # *64 keeps fp8(v) away from the subnormal range; the kernel divides
    # it back out via the 64.0 in the row-sum transpose matmul
    v_f8 = np.ascontiguousarray(values * 64.0).astype(f8)

    maps = []
    for ci in range(n_cores):
        xc = xf[ci * R : (ci + 1) * R]
        x2c = (xc * xc).sum(1, dtype=np.float32)
        aux = np.empty((P, R + 2 * NT), np.float32)
        aux[:, :R] = x2c[None, :]
        aux[:, R : R + NT] = p2_pt
        aux[:, R + NT : R + 2 * NT] = nit_pt
        maps.append(
            dict(
                xT=np.ascontiguousarray(xc.T).astype(f8),
                posTm2=posTm2,
                v=v_f8,
                aux=aux,
            )
        )
    return maps


_prog_cache = {}


def get_program(uniform_nit=None):
    key = ("nc", uniform_nit)
    if key not in _prog_cache:
        _prog_cache[key] = build_program(uniform_nit=uniform_nit)
    return _prog_cache[key]


def kernel(x, positions, values, temperature):
    from concourse.bass_utils import run_bass_kernel_spmd

    temperature = np.asarray(temperature, np.float32)
    t0 = float(temperature.flat[0])
    uniform_nit = None
    if np.all(temperature == t0):
        uniform_nit = -1.0 / ((abs(t0) + 0.1) * 0.05)

    maps = prepare_in_maps(x, positions, values, temperature)
    nc = get_program(uniform_nit)
    res = run_bass_kernel_spmd(nc, maps, list(range(N_CORES)))
    B, T, D = np.asarray(x).shape
    out = np.concatenate(
        [np.asarray(res.results[i]["out"]) for i in range(N_CORES)], axis=0
    )
    return np.ascontiguousarray(out.reshape(B, T, D)).astype(np.float32)
